# revision 7
# baseline (speedup 1.0000x reference)
"""Trainium2 Bass kernel for a GNN NodeBlock:

    agg = segment_sum(edge_feat, recv_idx, num_segments=N)   # [N, d]
    out = concat([node_feat, agg], -1) @ W + b               # [N, d]

Distribution strategy (8 NeuronCores, no collectives needed):
  * Core i owns the node range [i*1250, (i+1)*1250). The host buckets
    edges by destination-node range, so each core receives exactly the
    edges that target its nodes and computes a COMPLETE aggregate for
    its slice. No cross-core reduction is required.
  * Within a core, edges are bucketed by 128-node chunk and padded to
    whole 128-edge blocks (pad rows have zero features, so they add 0).
  * On device, each 128-edge block is scatter-added with a one-hot
    matmul: DVE builds onehot[e, n] = (iota[n] == local_idx[e]) and the
    PE computes aggT[feat, node] += edge_blockT.T @ onehot, accumulating
    all blocks of a chunk into the same PSUM region.
  * The node GEMM runs on-chip in transposed layout (aggT is already
    transposed): outT = W_top.T @ node_featT + W_bot.T @ aggT + b.
  * Host work is layout-only: permutation/padding of inputs and a
    transpose of outputs. All FLOPs happen on device.
"""

import math

import numpy as np

N_CORES = 8
N_NODES = 10000
D = 128
NPC = N_NODES // N_CORES          # nodes per core (1250)
CHUNKS = math.ceil(NPC / 128)     # 128-node chunks per core (10)
G = 64                            # 128-edge blocks per DMA group (2 MiB in fp16)

TRACE = False
LAST = {"exec_time_ns": None, "results": None}

_prog_cache = {}


def _build_program(caps):
    """Build + compile the (shared SPMD) Bass program for per-chunk block
    capacities `caps` (tuple of CHUNKS ints)."""
    import concourse.bacc as bacc
    import concourse.mybir as mybir
    import concourse.tile as tile

    f32 = mybir.dt.float32
    f16 = mybir.dt.float16
    NB = sum(caps)
    S = 16  # blocks per one-hot compare group

    nc = bacc.Bacc(
        "TRN2",
        target_bir_lowering=False,
        debug=False,
        enable_asserts=False,
        num_devices=N_CORES,
    )

    edge_d = nc.dram_tensor("edge", [128, NB * D], f16, kind="ExternalInput")
    idx_d = nc.dram_tensor("idxb8", [128, NB * 8], f16, kind="ExternalInput")
    iota_d = nc.dram_tensor("iota", [128, S * 128], f16, kind="ExternalInput")
    nft_d = nc.dram_tensor("nfT", [128, NPC], f32, kind="ExternalInput")
    w_d = nc.dram_tensor("w", [2 * D, D], f32, kind="ExternalInput")
    b_d = nc.dram_tensor("b", [128, 1], f32, kind="ExternalInput")
    out_d = nc.dram_tensor("outT", [128, NPC], f32, kind="ExternalOutput")

    # (chunk, first, last) per block; bank b covers chunks 4b..4b+3
    blocks = []
    for c, cap in enumerate(caps):
        for k in range(cap):
            blocks.append((c, k == 0, k == cap - 1))
    last_block_of_bank = {}
    for i, (c, _f, last) in enumerate(blocks):
        if last and (c % 4 == 3 or c == CHUNKS - 1):
            last_block_of_bank[i] = c // 4

    # idxb8 quarter splits (block-count multiples of S)
    qblocks = ((NB + 63) // 64) * 16
    qs = [(q * qblocks, min((q + 1) * qblocks, NB)) for q in range(4)]
    qs = [(a, b) for a, b in qs if b > a]

    with tile.TileContext(nc) as tc:
        with (
            tc.tile_pool(name="consts", bufs=1) as cpool,
            tc.tile_pool(name="edges", bufs=3) as epool,
            tc.tile_pool(name="oh", bufs=4) as ohpool,
            tc.tile_pool(name="post", bufs=3) as ppool,
            tc.tile_pool(name="psum", bufs=1, space="PSUM") as pspool,
            tc.tile_pool(name="psum2", bufs=3, space="PSUM") as pspool2,
        ):
            # Constants ride the second HWDGE queue (ACT) so the edge
            # stream on the sync queue starts immediately.
            iota_t = cpool.tile([128, S * 128], f16)
            nc.scalar.dma_start(iota_t[:], iota_d[:])
            idx_t = cpool.tile([128, NB * 8], f16)
            for a, b_ in qs:
                nc.scalar.dma_start(idx_t[:, a * 8 : b_ * 8], idx_d[:, a * 8 : b_ * 8])
            wtop = cpool.tile([128, D], f32)
            nc.scalar.dma_start(wtop[:], w_d[0:128, :])
            wbot = cpool.tile([128, D], f32)
            nc.scalar.dma_start(wbot[:], w_d[128:256, :])
            bias = cpool.tile([128, 1], f32)
            nc.scalar.dma_start(bias[:], b_d[:])
            nft = cpool.tile([128, NPC], f32)
            nc.scalar.dma_start(nft[:], nft_d[:])

            # Phase 1: scatter-add all edge blocks into aggT (PSUM).
            aggT = pspool.tile([128, CHUNKS * 128], f32)

            def phase2_bank(bank):
                lo = bank * 512
                hi = min(lo + 512, NPC)
                w = hi - lo
                aggs = ppool.tile([128, 512], f32, name="aggs")
                nc.scalar.activation(
                    aggs[:, :w], aggT[:, lo:hi], mybir.ActivationFunctionType.Copy
                )
                outT = pspool2.tile([128, 512], f32, name="outT")
                nc.tensor.matmul(
                    outT[:, :w], wtop[:], nft[:, lo:hi], start=True, stop=False
                )
                nc.tensor.matmul(
                    outT[:, :w], wbot[:], aggs[:, :w], start=False, stop=True
                )
                res = ppool.tile([128, 512], f32, name="res")
                nc.scalar.activation(
                    res[:, :w],
                    outT[:, :w],
                    mybir.ActivationFunctionType.Identity,
                    bias=bias[:],
                )
                nc.sync.dma_start(out_d[:, lo:hi], res[:, :w])

            b_i = 0
            n_groups = (NB + G - 1) // G
            for g in range(n_groups):
                gg = min(G, NB - g * G)
                et = epool.tile([128, G * D], f16)
                if g == n_groups - 1:
                    # Split the last group's DMA for tail latency.
                    for cs in range(0, gg, S):
                        ce = min(cs + S, gg)
                        nc.sync.dma_start(
                            et[:, cs * D : ce * D],
                            edge_d[:, (g * G + cs) * D : (g * G + ce) * D],
                        )
                else:
                    nc.sync.dma_start(
                        et[:, : gg * D],
                        edge_d[:, g * G * D : (g * G + gg) * D],
                    )
                for s0 in range(0, gg, S):
                    ss = min(S, gg - s0)
                    base = g * G + s0
                    # One wide compare builds ss one-hots at once. in1 reads
                    # the 8x-replicated idx stream through a broadcast AP
                    # (packed 8-element runs keep the DVE 2x mode).
                    in1 = (
                        idx_t[:, base * 8 : (base + ss) * 8]
                        .rearrange("p (s r) -> p s r", r=8)
                        .unsqueeze(2)
                        .broadcast_to([128, ss, 16, 8])
                    )
                    oh = ohpool.tile([128, S * 128], f16, name="oh")
                    nc.vector.tensor_tensor(
                        out=oh[:, : ss * 128].rearrange(
                            "p (s q r) -> p s q r", q=16, r=8
                        ),
                        in0=iota_t[:, : ss * 128].rearrange(
                            "p (s q r) -> p s q r", q=16, r=8
                        ),
                        in1=in1,
                        op=mybir.AluOpType.is_equal,
                    )
                    for s in range(s0, s0 + ss):
                        c, first, last = blocks[b_i]
                        nc.tensor.matmul(
                            aggT[:, c * 128 : (c + 1) * 128],
                            et[:, s * D : (s + 1) * D],
                            oh[:, (s - s0) * 128 : (s - s0 + 1) * 128],
                            start=first,
                            stop=last,
                        )
                        # Phase 2 for a PSUM bank as soon as its chunks done.
                        if b_i in last_block_of_bank:
                            phase2_bank(last_block_of_bank[b_i])
                        b_i += 1

    nc.compile()
    return nc


def _prep(edge_feat, node_feat, recv_idx, W, b):
    """Bucket + pad edges per (core, chunk); build per-core input maps."""
    edge_feat = np.ascontiguousarray(np.asarray(edge_feat, dtype=np.float32))
    node_feat = np.ascontiguousarray(np.asarray(node_feat, dtype=np.float32))
    idx = np.asarray(recv_idx).astype(np.int64)
    W = np.ascontiguousarray(np.asarray(W, dtype=np.float32))
    b = np.ascontiguousarray(np.asarray(b, dtype=np.float32).reshape(D, 1))

    core = idx // NPC
    lnode = idx - core * NPC
    chunk = lnode >> 7
    within = (lnode & 127).astype(np.float32)  # cast to fp16 after sort

    key = core * CHUNKS + chunk
    order = np.argsort(key, kind="stable")
    counts = np.bincount(key, minlength=N_CORES * CHUNKS).reshape(N_CORES, CHUNKS)
    caps = tuple(
        max(1, int(math.ceil(counts[:, c].max() / 128.0))) for c in range(CHUNKS)
    )
    NB = sum(caps)

    sorted_feat = edge_feat[order].astype(np.float16)
    sorted_within = within[order].astype(np.float16)
    run_starts = np.concatenate([[0], np.cumsum(counts.reshape(-1))]).astype(np.int64)
    slot_starts = np.concatenate([[0], np.cumsum(np.array(caps))]) * 128

    iota = np.ascontiguousarray(
        np.tile(np.arange(128, dtype=np.float16), (128, 16))
    )

    in_maps = []
    for co in range(N_CORES):
        pf = np.zeros((NB * 128, D), dtype=np.float16)
        pi = np.zeros((NB * 128,), dtype=np.float16)
        for c in range(CHUNKS):
            k = co * CHUNKS + c
            r0, r1 = run_starts[k], run_starts[k + 1]
            s0 = slot_starts[c]
            pf[s0 : s0 + (r1 - r0)] = sorted_feat[r0:r1]
            pi[s0 : s0 + (r1 - r0)] = sorted_within[r0:r1]
        # Partition-major layout: SBUF partition p holds, for every block,
        # the feature row of that block's lane-p edge (contiguous per
        # partition -> clean large DMA descriptors).
        edge_in = np.ascontiguousarray(
            pf.reshape(NB, 128, D).transpose(1, 0, 2).reshape(128, NB * D)
        )
        idx_in = np.ascontiguousarray(
            np.repeat(pi.reshape(NB, 128).T, 8, axis=1)
        )
        nft_in = np.ascontiguousarray(node_feat[co * NPC : (co + 1) * NPC].T)
        in_maps.append(
            {
                "edge": edge_in,
                "idxb8": idx_in,
                "iota": iota,
                "nfT": nft_in,
                "w": W,
                "b": b,
            }
        )
    return caps, in_maps


def kernel(**inputs):
    from concourse.bass_utils import run_bass_kernel_spmd

    caps, in_maps = _prep(
        inputs["edge_feat"],
        inputs["node_feat"],
        inputs["recv_idx"],
        inputs["W"],
        inputs["b"],
    )
    nc = _prog_cache.get(caps)
    if nc is None:
        nc = _prog_cache.setdefault(caps, _build_program(caps))

    res = run_bass_kernel_spmd(nc, in_maps, list(range(N_CORES)), trace=TRACE)
    LAST["exec_time_ns"] = res.exec_time_ns
    LAST["results"] = res

    out = np.empty((N_NODES, D), dtype=np.float32)
    for co in range(N_CORES):
        out[co * NPC : (co + 1) * NPC] = res.results[co]["outT"].T
    return out
